# revision 32
# baseline (speedup 1.0000x reference)
"""Trainium2 Bass kernel for MultiHeadAttention with relative-position bias.

Problem shapes: N=4, S=1024, H=1024, NH=16, D=64, P=20 (clamp window).
Returns (out, ctx) like the reference.

Sharding: 8 cores; core c handles batch n=c//2, head-group hg=c%2 (8 heads).
Each core computes its heads' QKV projections, attention, the ctx column
slice, and a partial out (row-sharded Wo contraction). Host sums the two
partials per batch and adds bo' (bo + bv@Wo; bv is folded out of the device
V path and re-added to ctx on the host).

Device-side structure (v3, "transposed scores", q-halves):
  - Scores are computed TRANSPOSED: for each k-block kb (128 keys on
    partitions) and head h, S^T[k, q] = K_kb Q^T over the causally-valid
    q range. The exp'd tile P^T feeds the AV matmul directly as the moving
    operand, so the P-transpose stage of v1 (36K PE cycles + 80 PSUM
    evictions) disappears.
  - Softmax denominators ride for free: V gets a constant ones column per
    head (vN65 layout [k, kb, 65, h]), so the AV accumulation produces
    ctx^T rows 0:64 and the softmax sum in row 64 of the same PSUM group.
  - The relative-position band + causal mask stay in q-layout staging
    tiles (template + diagonal-DMA of B-values), folded into the scores
    via transpose-style matmuls (stg as lhsT, identity as rhs) inside the
    scores' PSUM accumulation group. A 45-column zero pad lets the 19-wide
    diagonal "sliver" piece respect PE base-partition alignment (64).
  - Band values B = 8*(E(j) - e40) . Q come straight from a matmul against
    a host-precomputed rel8 table.
  - Work is organized in q-HALVES (512 q) x heads: each (half, head,
    k-block) score tile is a full PSUM bank [128, 512] f32 (hardware
    requires score sub-tiles not to share a bank), exp'd in one 2D ACT op
    over the causally-valid span. Output stages (normalize + ctx store +
    out-projection partial) lag one half and interleave with the next
    half's chains.
"""

import sys

if "/opt/trn_rl_repo" not in sys.path:
    sys.path.insert(0, "/opt/trn_rl_repo")

import numpy as np

import concourse.bass as bass
import concourse.mybir as mybir
import concourse.tile as tile
from concourse import bacc
from concourse.bass_utils import run_bass_kernel_spmd

F32 = mybir.dt.float32
F32R = mybir.dt.float32r
BF16 = mybir.dt.bfloat16
F8 = mybir.dt.float8e4
PM_DR = mybir.MatmulPerfMode.DoubleRow
AF = mybir.ActivationFunctionType

S = 1024
D = 64
NHG = 8      # heads per core
NPAIR = 4    # head pairs per core
HC = 8       # 128-row contraction chunks over H
MASKV = -1.0e9
W2 = 192     # staging window: 45 zero-pad + 19 + 128
PAD = 45
NT = 4       # k-blocks (staging tiles) per half

# packed-constant column layout (bf16 elements)
C_TEMPL = 0
C_IDENT = C_TEMPL + W2                # 192
C_REL8 = C_IDENT + 128                # 320
C_SEL = C_REL8 + 20                   # 340 (4 x [32, 512], rows 0:32)
C_BQ = C_SEL + 4 * 512                # 2388
C_BK = C_BQ + 8
C_END = C_BK + 8


def build_nc():
    nc = bacc.Bacc("TRN2", target_bir_lowering=False, debug=False)

    xq8 = nc.dram_tensor("xq8", (128, 4, 2, S), F8,
                         kind="ExternalInput").ap()
    xk8 = nc.dram_tensor("xk8", (128, 4, 2, S), F8,
                         kind="ExternalInput").ap()
    xvT = nc.dram_tensor("xvT", (S, S), BF16, kind="ExternalInput").ap()
    wq8 = nc.dram_tensor("wq8", (128, 4, 2, 512), F8,
                         kind="ExternalInput").ap()
    wk8 = nc.dram_tensor("wk8", (128, 4, 2, 512), F8,
                         kind="ExternalInput").ap()
    wv = nc.dram_tensor("wv", (S, 512), BF16, kind="ExternalInput").ap()
    wo = nc.dram_tensor("wo", (512, S), BF16, kind="ExternalInput").ap()
    cst = nc.dram_tensor("cst", (128, C_END), BF16,
                         kind="ExternalInput").ap()

    o_part = nc.dram_tensor("o_part", (S, S), BF16,
                            kind="ExternalOutput").ap()
    ctx_out = nc.dram_tensor("ctx_out", (S, 512), BF16,
                             kind="ExternalOutput").ap()

    # greedy ACT/DVE balance for PSUM->SBUF evictions. Pre-loaded with the
    # fixed per-engine work (ACT: exps ~44us; DVE: cn mults ~7us).
    ebusy = {"act": 44000.0, "dve": 7000.0}

    def _pick(cact, cdve):
        if ebusy["act"] + cact < ebusy["dve"] + cdve:
            ebusy["act"] += cact
            return "act"
        ebusy["dve"] += cdve
        return "dve"

    def ecopy(out, in_, cols, bf=False):
        if _pick(cols * 0.833 + 450.0,
                 cols * (0.521 if bf else 1.042) + 295.0) == "act":
            nc.scalar.copy(out, in_)
        else:
            nc.vector.tensor_copy(out, in_)

    def ebias(out, in_, bias, cols):
        if _pick(cols * 0.833 + 450.0, cols * 1.042 + 295.0) == "act":
            nc.scalar.activation(out, in_, AF.Identity, bias=bias)
        else:
            nc.vector.tensor_scalar_add(out, in_, bias)

    def ebias_s(out, in_, bias, cols, force=None):
        # out = in/16 + bias (un-scales the x16 fp8 weight scaling)
        if force:
            eng = force
            ebusy[eng] += (cols * 0.833 + 450.0 if eng == "act"
                           else cols * 1.042 + 295.0)
        else:
            eng = _pick(cols * 0.833 + 450.0, cols * 1.042 + 295.0)
        if eng == "act":
            nc.scalar.activation(out, in_, AF.Identity, bias=bias,
                                 scale=1.0 / 16.0)
        else:
            nc.vector.tensor_scalar(out, in_, 1.0 / 16.0, bias,
                                    mybir.AluOpType.mult,
                                    mybir.AluOpType.add)

    with tile.TileContext(nc) as tc:
        import contextlib

        with contextlib.ExitStack() as ctx:
            ep = ctx.enter_context

            cpool = ep(tc.tile_pool(name="consts", bufs=1))
            xTp = ep(tc.tile_pool(name="xTp", bufs=2))
            wxp = ep(tc.tile_pool(name="wxp", bufs=2))
            wop = ep(tc.tile_pool(name="wop", bufs=1))
            xvp = ep(tc.tile_pool(name="xvp", bufs=1))
            wvp = ep(tc.tile_pool(name="wvp", bufs=1))
            big = ep(tc.tile_pool(name="big", bufs=1))
            stgp = ep(tc.tile_pool(name="stgp", bufs=17))
            srcp = ep(tc.tile_pool(name="srcp", bufs=8))
            ptp = ep(tc.tile_pool(name="ptp", bufs=6))
            cujp = ep(tc.tile_pool(name="cujp", bufs=2))
            smp = ep(tc.tile_pool(name="smp", bufs=2))
            cnp = ep(tc.tile_pool(name="cnp", bufs=2))
            ctp = ep(tc.tile_pool(name="ctp", bufs=2))
            osb = ep(tc.tile_pool(name="osb", bufs=2))

            # PSUM: 3 (scores) + 3 (ctx accum) + 2 (general) = 8 banks
            spp = ep(tc.tile_pool(name="spp", bufs=3, space="PSUM"))
            cxp = ep(tc.tile_pool(name="cxp", bufs=2, space="PSUM"))
            gpp = ep(tc.tile_pool(name="gpp", bufs=3, space="PSUM"))

            # ---------------- loads ---------------------------------------
            C = cpool.tile([128, C_END], BF16, tag="cst", name="C")[:]
            templ2 = C[:, C_TEMPL:C_TEMPL + W2]
            identb = C[:, C_IDENT:C_IDENT + 128]
            rel8 = C[:, C_REL8:C_REL8 + 20]
            sels = [C[0:32, C_SEL + 512 * t:C_SEL + 512 * (t + 1)]
                    for t in range(4)]
            bq_sb = C[:, C_BQ:C_BQ + 8].bitcast(F32)
            bk_sb = C[:, C_BK:C_BK + 8].bitcast(F32)

            # load order follows PE consumption: weights first, then x in
            # column-halves (projection qc0 only needs cols 0:512 of every
            # chunk); xv streams in column-halves matching v_chain order.
            xTq = xTp.tile([128, 4, 2, S], F8, tag="xT", name="xTq")[:]
            w_q = wxp.tile([128, 4, 2, 512], F8, tag="wx", name="w_q")[:]
            nc.sync.dma_start(w_q[:, 0:1, :, :], wq8[:, 0:1, :, :])
            nc.sync.dma_start(xTq[:, 0:1, :, 0:512], xq8[:, 0:1, :, 0:512])
            nc.sync.dma_start(w_q[:, 1:2, :, :], wq8[:, 1:2, :, :])
            nc.sync.dma_start(xTq[:, 1:2, :, 0:512], xq8[:, 1:2, :, 0:512])
            nc.sync.dma_start(w_q[:, 2:4, :, :], wq8[:, 2:4, :, :])
            nc.sync.dma_start(xTq[:, 2:4, :, 0:512], xq8[:, 2:4, :, 0:512])
            nc.sync.dma_start(xTq[:, :, :, 512:1024],
                              xq8[:, :, :, 512:1024])
            nc.sync.dma_start(C, cst)

            # V inputs before K: PE order is Q -> V-chains 0-3 -> K
            xTv = xvp.tile([128, HC, S], BF16, tag="xv", name="xTv")[:]
            w_v = wvp.tile([128, HC, 512], BF16, tag="wv", name="w_v")[:]
            wvre = wv.rearrange("(c p) n -> p c n", p=128)
            xvre = xvT.rearrange("(c p) n -> p c n", p=128)
            nc.sync.dma_start(w_v[:], wvre)
            nc.sync.dma_start(xTv[:, :, 0:512], xvre[:, :, 0:512])

            xTk = xTp.tile([128, 4, 2, S], F8, tag="xT", name="xTk")[:]
            w_k = wxp.tile([128, 4, 2, 512], F8, tag="wx", name="w_k")[:]
            nc.sync.dma_start(w_k[:], wk8)
            nc.sync.dma_start(xTk[:, :, :, 0:512], xk8[:, :, :, 0:512])
            nc.sync.dma_start(xTv[:, :, 512:1024], xvre[:, :, 512:1024])
            nc.sync.dma_start(xTk[:, :, :, 512:1024],
                              xk8[:, :, :, 512:1024])

            qT = big.tile([128, NPAIR, S], BF16, tag="qT", name="qT")[:]
            kT = big.tile([128, NPAIR, S], BF16, tag="kT", name="kT")[:]
            # vN65[k, kb, d(65), h]; row d=64 is the ones column
            vN = big.tile([128, HC, 65, NHG], BF16, tag="vN", name="vN")[:]

            def proj_qk(xT, w_sb, outT, b_sb, qc=None):
                # fp8 DoubleRow: each matmul contracts 256 rows (2 k-tiles)
                for qc in ((0, 1) if qc is None else (qc,)):
                    for pair in range(NPAIR):
                        pp = gpp.tile([128, 512], F32, tag="gp", name="pp")
                        for c2 in range(4):
                            nc.tensor.matmul(
                                pp[:],
                                w_sb[:, c2, :, pair * 128:(pair + 1) * 128],
                                xT[:, c2, :, qc * 512:(qc + 1) * 512],
                                start=(c2 == 0), stop=(c2 == 3),
                                perf_mode=PM_DR)
                        ebias_s(outT[:, pair, qc * 512:(qc + 1) * 512],
                                pp[:], b_sb[:, pair:pair + 1], 512,
                                force=("act", "dve")[pair % 2])

            def v_chain(kb):
                pp = gpp.tile([128, NHG, 64], F32, tag="gp", name="pp")
                for hc in range(HC):
                    nc.tensor.matmul(
                        pp[:, :, :],
                        xTv[:, hc, kb * 128:(kb + 1) * 128],
                        w_v[:, hc, :],
                        start=(hc == 0), stop=(hc == HC - 1))
                ecopy(vN[:, kb, 0:64, :].transpose([0, 2, 1]),
                      pp[:], 512)

            # band pre-pass for half J: staging tiles for k-blocks
            # t = 4J .. 4J+3 of every head.
            stg_tiles = {}

            def prepass(J):
                for h in range(NHG):
                    pair, half = divmod(h, 2)
                    stg = stgp.tile([128, NT, W2], BF16, tag="stg",
                                    name=f"stg{h}_{J}")[:]
                    stg_tiles[(h, J)] = stg
                    for t in range(NT):
                        nc.gpsimd.tensor_copy(stg[:, t, :], templ2)
                    bp = gpp.tile([128, NT, 64], F32, tag="gp", name="bp")
                    for t in range(NT):
                        nc.tensor.matmul(
                            bp[:, t, 0:20],
                            qT[64 * half:64 * half + 64, pair,
                               (NT * J + t) * 128:(NT * J + t + 1) * 128],
                            rel8[64 * half:64 * half + 64, :],
                            start=True, stop=True)
                    srcb = srcp.tile([128, NT, 20], BF16, tag="srcb",
                                     name="srcb")
                    ecopy(srcb[:], bp[:, :, 0:20], 80)
                    diag = bass.AP(
                        stg.tensor, stg.offset + PAD,
                        [[NT * W2 + 1, 128], [W2, NT], [1, 20]])
                    nc.sync.dma_start(diag, srcb[:])

            # ------ attention: unified pipeline over (head, k-block) ------
            # One software pipeline per half: S two units ahead of AV, so
            # chain boundaries cost nothing; `hooks` inject output-stage /
            # prepass pieces every few units to fill ACT-latency slack.
            def half_pass(J, hooks, rjTs):
                nkb = NT * J + NT
                units = [(h, kb) for h in range(NHG) for kb in range(nkb)]
                sps = {}
                pts = {}
                cxs = {}

                def s_u(i):
                    h, kb = units[i]
                    pair, half = divmod(h, 2)
                    qTh = qT[64 * half:64 * half + 64, pair,
                             512 * J:512 * (J + 1)]
                    lhs = kT[64 * half:64 * half + 64, pair,
                             kb * 128:(kb + 1) * 128]
                    stgj = stg_tiles[(h, J)]
                    sp = spp.tile([128, 512], F32, tag="sp", name="sp")
                    bs = 128 * kb - 512 * J
                    if kb < NT * J - 1:
                        nc.tensor.matmul(sp[:, 0:512], lhs, qTh,
                                         start=True, stop=True)
                    elif kb == NT * J - 1:
                        # sliver-only tile (band tail into stg_{4J})
                        nc.tensor.matmul(sp[:, 0:19], lhs, qTh[:, 0:19],
                                         start=True, stop=False)
                        nc.tensor.matmul(sp[64:128, 0:19],
                                         stgj[0:19, 0, 0:64],
                                         identb[0:19, 0:19],
                                         start=False, stop=False)
                        nc.tensor.matmul(sp[:, 19:512], lhs, qTh[:, 19:512],
                                         start=False, stop=True)
                    elif kb < NT * J + NT - 1:
                        # piece1 + sliver (sliver rides post-stop with
                        # skip_group_check; its bytes are non-pending)
                        ti = kb - NT * J
                        nc.tensor.matmul(sp[:, bs:bs + 128], lhs,
                                         qTh[:, bs:bs + 128],
                                         start=True, stop=False)
                        nc.tensor.matmul(sp[:, bs:bs + 128],
                                         stgj[:, ti, 64:192], identb,
                                         start=False, stop=False)
                        nc.tensor.matmul(sp[:, bs + 128:512], lhs,
                                         qTh[:, bs + 128:512],
                                         start=False, stop=True)
                        nc.tensor.matmul(sp[64:128, bs + 128:bs + 147],
                                         stgj[0:19, ti + 1, 0:64],
                                         identb[0:19, 0:19],
                                         start=False, stop=True,
                                         skip_group_check=True)
                    else:  # kb == 4J+3: last diagonal block, piece1 only
                        nc.tensor.matmul(sp[:, 384:512], lhs,
                                         qTh[:, 384:512],
                                         start=True, stop=False)
                        nc.tensor.matmul(sp[:, 384:512],
                                         stgj[:, 3, 64:192], identb,
                                         start=False, stop=True)
                    sps[i] = sp

                def e_u(i):
                    h, kb = units[i]
                    vs = max(0, 128 * kb - 512 * J)
                    pt = ptp.tile([128, 512], BF16, tag="pt", name="pt")
                    nc.scalar.activation(pt[:, vs:512],
                                         sps.pop(i)[:, vs:512],
                                         AF.Exp, scale=1.0 / 64.0)
                    pts[i] = pt

                def av_u(i):
                    h, kb = units[i]
                    vs = max(0, 128 * kb - 512 * J)
                    if kb == 0:
                        cxs[h] = cxp.tile([65, 512], F32, tag="cx",
                                          name="cxh")
                    nc.tensor.matmul(
                        cxs[h][0:65, vs:512], vN[:, kb, :, h],
                        pts.pop(i)[:, vs:512],
                        start=(kb == 0), stop=(kb == nkb - 1))
                    if kb == nkb - 1:
                        ecopy(cuj[:, h, :], cxs.pop(h)[0:65, 0:512],
                              512, bf=False)
                        # gather this head's softmax sums into rjT layout
                        # (rows 4h..4h+4 = contiguous so deps track right)
                        nc.sync.dma_start(rjTs[4 * h:4 * h + 4, :],
                                          cuj[64:65, h, :])

                n = len(units)
                s_u(0)
                s_u(1)
                e_u(0)
                for i in range(n - 2):
                    s_u(i + 2)
                    e_u(i + 1)
                    if i in hooks:
                        hooks[i]()
                    av_u(i)
                e_u(n - 1)
                av_u(n - 2)
                av_u(n - 1)

            # rjb: per-(q,h) reciprocal softmax sums broadcast to [128, 512]
            # sums row (partition 64 of cuj) is gathered to [32,128] layout
            # by a strided SBUF->SBUF DMA, then reciprocal + sel broadcast.
            def rjb_ab(J, rjTs, st, pool=None):
                rjT = smp.tile([32, 128], BF16, tag="rjT", name="rjT")
                with nc.allow_low_precision(reason="bf16 softmax sums"):
                    nc.vector.reciprocal(rjT[:], rjTs[:])
                rjb = smp.tile([128, 4, 512], BF16, tag="rjb", name="rjb")
                for tt in range(4):
                    rb = (pool or gpp).tile([128, 512], F32,
                                            tag="sp" if pool else "gp",
                                            name="rb")
                    nc.tensor.matmul(
                        rb[:], rjT[:], sels[tt],
                        start=True, stop=True)
                    ecopy(rjb[:, tt, :], rb[:], 512, bf=True)
                st["rjb"] = rjb

            # output stage pieces for half J, q-block tt (0..3)
            def out_a1(J, tt, cuj_j, st):
                cnall = gpp.tile([128, 512], BF16, tag="gp", name="cnall")
                for h in range(NHG):
                    nc.tensor.transpose(
                        cnall[:, h * 64:(h + 1) * 64],
                        cuj_j[0:64, h, tt * 128:(tt + 1) * 128],
                        identb[0:64, 0:64])
                st[f"cnall{tt}"] = cnall

            def out_a2(J, tt, st):
                rjb = st["rjb"]
                cnall = st.pop(f"cnall{tt}")
                cn = cnp.tile([128, 512], BF16, tag="cn")
                nc.vector.tensor_tensor(cn[:], cnall[:], rjb[:, tt, :],
                                        mybir.AluOpType.mult)
                st[f"cn{tt}"] = cn

            def out_a(J, tt, cuj_j, st):
                out_a1(J, tt, cuj_j, st)
                out_a2(J, tt, st)

            def out_b(J, tt, st):
                cn = st.pop(f"cn{tt}")
                qb = 4 * J + tt
                nc.sync.dma_start(
                    ctx_out[qb * 128:(qb + 1) * 128, :], cn[:])
                rt = gpp.tile([128, 512], BF16, tag="gp", name="rt")
                for pc in range(NPAIR):
                    nc.tensor.transpose(
                        rt[:, pc * 128:(pc + 1) * 128],
                        cn[:, pc * 128:(pc + 1) * 128],
                        identb)
                ctxT = ctp.tile([128, NPAIR, 128], BF16, tag="ctxT")
                ecopy(ctxT[:], rt[:], 512, bf=True)
                st[f"ctxT{tt}"] = ctxT

            def out_c(J, tt, st, split_dma=False):
                ctxT = st.pop(f"ctxT{tt}")
                qb = 4 * J + tt
                ou = osb.tile([128, 1024], BF16, tag="ou")
                for oc in range(2):
                    op = gpp.tile([128, 512], F32, tag="gp", name="op")
                    for pc in range(NPAIR):
                        nc.tensor.matmul(
                            op[:],
                            ctxT[:, pc, :],
                            wo_sb[:, pc, oc * 512:(oc + 1) * 512],
                            start=(pc == 0), stop=(pc == NPAIR - 1))
                    ecopy(ou[:, oc * 512:(oc + 1) * 512], op[:], 512)
                    if split_dma:
                        nc.sync.dma_start(
                            o_part[qb * 128:(qb + 1) * 128,
                                   oc * 512:(oc + 1) * 512],
                            ou[:, oc * 512:(oc + 1) * 512])
                if not split_dma:
                    nc.sync.dma_start(o_part[qb * 128:(qb + 1) * 128, :],
                                      ou[:])

            # ---------------- flat schedule -------------------------------
            nc.vector.memset(vN[:, :, 64, :], 1.0)
            proj_qk(xTq, w_q, qT, bq_sb)
            prepass(0)
            for kb in range(4):
                v_chain(kb)
            proj_qk(xTk, w_k, kT, bk_sb)
            wo_sb = wop.tile([128, NPAIR, S], BF16, tag="wo",
                             name="wo_sb")[:]
            nc.sync.dma_start(wo_sb,
                              wo.rearrange("(c p) n -> p c n", p=128))

            cuj = cujp.tile([65, NHG, 512], BF16, tag="cuj", name="cuj0")[:]
            cujs = {0: cuj}
            rjTs0 = smp.tile([32, 128], BF16, tag="rjTs", name="rjTs0")
            rjTs1 = smp.tile([32, 128], BF16, tag="rjTs", name="rjTs1")
            # half 0: 32 units; v_chains 4-7 and prepass(1) as PE filler
            half_pass(0, {6: lambda: prepass(1), 14: lambda: v_chain(4),
                          18: lambda: v_chain(5), 22: lambda: v_chain(6),
                          26: lambda: v_chain(7)}, rjTs0)
            cuj = cujp.tile([65, NHG, 512], BF16, tag="cuj", name="cuj1")[:]
            cujs[1] = cuj
            # half 1: 64 units; half-0 output stage woven in
            st0 = {}
            pieces = [
                lambda: rjb_ab(0, rjTs0, st0),
                lambda: out_a(0, 0, cujs[0], st0),
                lambda: out_b(0, 0, st0),
                lambda: out_c(0, 0, st0),
                lambda: out_a(0, 1, cujs[0], st0),
                lambda: out_b(0, 1, st0),
                lambda: out_c(0, 1, st0),
                lambda: out_a(0, 2, cujs[0], st0),
                lambda: out_b(0, 2, st0),
                lambda: out_c(0, 2, st0),
                lambda: out_a(0, 3, cujs[0], st0),
                lambda: out_b(0, 3, st0),
                lambda: out_c(0, 3, st0),
            ]
            hooks1 = {3 + round(i * 55 / 12): fn
                      for i, fn in enumerate(pieces)}
            half_pass(1, hooks1, rjTs1)

            # tail: output stage for half 1, pieces staggered so each PE
            # piece's upstream DVE/ACT step is already in flight.
            st1 = {}
            out_a1(1, 0, cujs[1], st1)
            out_a1(1, 1, cujs[1], st1)
            rjb_ab(1, rjTs1, st1, pool=spp)
            out_a2(1, 0, st1)
            out_b(1, 0, st1)
            out_a1(1, 2, cujs[1], st1)
            out_a2(1, 1, st1)
            out_b(1, 1, st1)
            out_c(1, 0, st1)
            out_a1(1, 3, cujs[1], st1)
            out_a2(1, 2, st1)
            out_b(1, 2, st1)
            out_c(1, 1, st1)
            out_a2(1, 3, st1)
            out_b(1, 3, st1)
            out_c(1, 2, st1)
            out_c(1, 3, st1, split_dma=True)

    nc.compile()
    return nc


_NC = None


def _get_nc():
    global _NC
    if _NC is None:
        _NC = build_nc()
    return _NC


def make_in_maps(query, key, value, Wq, bq, Wk, bk, Wv, bv, Wo, rel_emb):
    import ml_dtypes
    BF = ml_dtypes.bfloat16
    NF8 = mybir.dt.np(F8)
    asf = lambda a: np.ascontiguousarray(a, dtype=np.float32)
    asb = lambda a: np.ascontiguousarray(np.asarray(a, np.float32),
                                         dtype=BF)

    def dr8(a, scale=1.0):
        # [1024, M] -> DoubleRow layout [128 p, 4 c2, 2 i, M] in fp8
        a = np.asarray(a, np.float32) * scale
        return np.ascontiguousarray(
            a.reshape(4, 2, 128, -1).transpose(2, 0, 1, 3).astype(NF8))

    # rel8[d, j] = 8*(rel_emb[39-j] - rel_emb[40]), stacked twice over d
    re = np.asarray(rel_emb, np.float32)
    r8 = 8.0 * (re[39::-1][:20].T - re[40][:, None])  # [64, 20]
    rel8 = np.concatenate([r8, r8], axis=0).astype(BF)  # [128, 20]

    # templ2: [128, W2]; col c masks MASKV where c >= p+PAD+20
    templ2 = np.zeros((128, W2), np.float32)
    for p in range(128):
        templ2[p, p + PAD + 20:] = MASKV
    templ2 = templ2.astype(BF)

    ident = np.eye(128, dtype=np.float32).astype(BF)

    # rjT rows are (4h + tt): head h's sums occupy rows 4h..4h+4
    sel = np.zeros((4, 32, 512), np.float32)
    for tt in range(4):
        for h in range(8):
            sel[tt, 4 * h + tt, h * 64:(h + 1) * 64] = 1.0
    sel = sel.astype(BF)

    # far-field rel-pos bias folded into bk
    bk_f = np.asarray(bk, np.float32) + np.tile(re[40] * 8.0, 16)

    in_maps = []
    for c in range(8):
        n, hg = divmod(c, 2)
        cs = slice(512 * hg, 512 * (hg + 1))
        cpack = np.zeros((128, C_END), BF)
        cpack[:, C_TEMPL:C_TEMPL + W2] = templ2
        cpack[:, C_IDENT:C_IDENT + 128] = ident
        cpack[:, C_REL8:C_REL8 + 20] = rel8
        for tt in range(4):
            cpack[0:32, C_SEL + 512 * tt:C_SEL + 512 * (tt + 1)] = sel[tt]
        cv = cpack.view(np.uint16)
        cv[:, C_BQ:C_BQ + 8] = asf(np.asarray(bq)[cs]).reshape(
            4, 128).T.copy().view(np.uint16).reshape(128, 8)
        cv[:, C_BK:C_BK + 8] = asf(bk_f[cs]).reshape(
            4, 128).T.copy().view(np.uint16).reshape(128, 8)
        in_maps.append({
            "xq8": dr8(np.asarray(query[n]).T),
            "xk8": dr8(np.asarray(key[n]).T),
            "xvT": asb(np.asarray(value[n]).T),
            "wq8": dr8(Wq[:, cs], 16.0),
            "wk8": dr8(Wk[:, cs], 16.0),
            "wv": asb(Wv[:, cs]),
            "wo": asb(Wo[cs, :]),
            "cst": cpack,
        })
    return in_maps


def run(inputs, trace=False, trace_kwargs=None):
    nc = _get_nc()
    in_maps = make_in_maps(
        np.asarray(inputs["query"]), np.asarray(inputs["key"]),
        np.asarray(inputs["value"]), np.asarray(inputs["Wq"]),
        np.asarray(inputs["bq"]), np.asarray(inputs["Wk"]),
        np.asarray(inputs["bk"]), np.asarray(inputs["Wv"]),
        np.asarray(inputs["bv"]), np.asarray(inputs["Wo"]),
        np.asarray(inputs["rel_emb"]))
    kw = {}
    if trace:
        kw["trace"] = True
        if trace_kwargs:
            kw.update(trace_kwargs)
    res = run_bass_kernel_spmd(nc, in_maps, core_ids=list(range(8)), **kw)
    bv = np.asarray(inputs["bv"], dtype=np.float32)
    Wo = np.asarray(inputs["Wo"], dtype=np.float32)
    bo = np.asarray(inputs["bo"], dtype=np.float32) + bv @ Wo
    out = np.zeros((4, S, S), np.float32)
    ctxf = np.zeros((4, S, S), np.float32)
    for c in range(8):
        n, hg = divmod(c, 2)
        out[n] += res.results[c]["o_part"]
        ctxf[n][:, 512 * hg:512 * (hg + 1)] = (
            res.results[c]["ctx_out"] + bv[512 * hg:512 * (hg + 1)])
    out += bo
    return (out, ctxf), res


def kernel(**inputs):
    (out, ctx), _ = run(inputs)
    return (out, ctx)
